# revision 7
# baseline (speedup 1.0000x reference)
"""Trainium2 Bass kernel for ConvertedLlamaAttention (LoRA q/k/v + RoPE + causal attention + out-proj).

Strategy: tensor-parallel over heads across 8 NeuronCores (4 heads/core).
All device matmuls run in "transposed" layouts so no on-device transposes are
needed anywhere:
  - Q^T, K^T computed as W^T-stationary matmuls (head_dim on partitions),
  - V computed in natural layout (seq on partitions) from the same X^T tiles,
  - scores computed transposed (S^T = K^T^T-slices @ Q^T); softmax denominators
    are accumulated on the vector engine (per-k-tile folds) and partition-
    reduced with a single all-ones matmul per (q-chunk, head) whose output is
    replicated across partitions (no separate broadcast matmul),
  - A*V consumes V in natural layout; out-proj consumes A*V^T as stationary.
All stationary matmul operands are bf16 (enables fast weight load); host-side
tensor layouts are per-partition contiguous so DMA descriptors are large.
Out-proj is interleaved into the attention stream (2 tiles per k-tile-pair) so
the tensor engine never starves and output DMA is spread across the kernel.
LoRA (incl. the half-interleave) is folded into the weights on the host.
Each core emits a partial (2048, 4096) bf16 output (row-parallel Wo); the host
sums in fp64.
"""
import sys

for _p in ("/opt/trn_rl_repo", "/root/.axon_site/_ro/trn_rl_repo"):
    if _p not in sys.path:
        sys.path.insert(0, _p)

from collections import deque

import numpy as np
import ml_dtypes

import concourse.bass as bass  # noqa: F401  (registers types)
import concourse.mybir as mybir
import concourse.tile as tile
from concourse import bacc, bass_utils

F32 = mybir.dt.float32
F32R = mybir.dt.float32r
BF16 = mybir.dt.bfloat16

H = 4096          # hidden
S = 2048          # sequence
P = 128           # partitions
HD = 128          # head dim
NCORES = 8
HPC = 4           # heads per core
CW = HPC * HD     # per-core width of q/k/v/attn dims = 512
NCHUNKS = 4       # seq chunks of 512
KCH = H // P      # 32 hidden chunks
LORA_SCALING = 2.0
EXP_SCALE = float(1.0 / np.sqrt(HD))

_CACHE = {}


def _build():
    nc = bacc.Bacc("TRN2", target_bir_lowering=False, debug=False, num_devices=NCORES)

    # Host-prearranged, per-partition-contiguous layouts.
    xt_d = nc.declare_dram_parameter("xt", [P, NCHUNKS, 4, 8, 512], BF16, isOutput=False)
    wq_d = nc.declare_dram_parameter("wq", [P, 8, 4, CW], BF16, isOutput=False)
    wk_d = nc.declare_dram_parameter("wk", [P, 8, 4, CW], BF16, isOutput=False)
    wv_d = nc.declare_dram_parameter("wv", [P, 8, 4, CW], BF16, isOutput=False)
    wot_d = nc.declare_dram_parameter("wot", [P, HPC, H], BF16, isOutput=False)
    cs_d = nc.declare_dram_parameter("cs", [P, S], F32, isOutput=False)
    masks_d = nc.declare_dram_parameter("masks", [P, 4, 512], F32R, isOutput=False)
    ones_d = nc.declare_dram_parameter("ones", [P, P], F32R, isOutput=False)
    out_d = nc.declare_dram_parameter("out", [S, H], BF16, isOutput=True)

    with tile.TileContext(nc) as tc:
        with tc.tile_pool(name="persist", bufs=1) as pp:
            qt = [pp.tile([P, S], BF16, tag=f"qt{h}", name=f"qt{h}") for h in range(HPC)]
            kt = [pp.tile([P, S], BF16, tag=f"kt{h}", name=f"kt{h}") for h in range(HPC)]
            v_sb = pp.tile([P, S // P, CW], F32R, tag="v")   # (128, 16, 512)
            cs_sb = pp.tile([P, S], F32, tag="cs")
            ones_sb = pp.tile([P, P], F32R, tag="ones")
            wot_sb = pp.tile([P, HPC, H], BF16, tag="wot")
            masks_sb = pp.tile([P, 4, 512], F32R, tag="masks")
            # Consts + wot preload on the scalar queue (idle until outputs);
            # projection weights own the sync queue, xt the gpsimd queue.
            nc.scalar.dma_start(cs_sb[:], cs_d[:])
            nc.scalar.dma_start(ones_sb[:], ones_d[:])
            nc.scalar.dma_start(masks_sb[:], masks_d[:])

            def rope(qp, dest, ncx):
                sl = slice(ncx * 512, (ncx + 1) * 512)
                t1 = ropep.tile([P, 512], F32, tag="r1")
                t2 = ropep.tile([P, 512], F32, tag="r2")
                # dest[0:64]  = q1*cos - q2*sin ; dest[64:] = q1*sin + q2*cos
                nc.vector.tensor_mul(t1[0:64], qp[0:64], cs_sb[0:64, sl])
                nc.vector.tensor_mul(t2[0:64], qp[64:128], cs_sb[64:128, sl])
                nc.vector.tensor_sub(dest[0:64], t1[0:64], t2[0:64])
                nc.vector.tensor_mul(t1[64:128], qp[0:64], cs_sb[64:128, sl])
                nc.vector.tensor_mul(t2[64:128], qp[64:128], cs_sb[0:64, sl])
                nc.vector.tensor_add(dest[64:128], t1[64:128], t2[64:128])

            # ---------------- Phase 1: Q^T, K^T, V projections ----------------
            with tc.tile_pool(name="xtp", bufs=6) as xtp, \
                 tc.tile_pool(name="wp", bufs=3) as wp, \
                 tc.tile_pool(name="ropep", bufs=2) as ropep, \
                 tc.tile_pool(name="projps", bufs=8, space="PSUM") as projps:
                for ncx in range(NCHUNKS):
                    xts = []
                    for b in range(4):
                        t = xtp.tile([P, 8, 512], BF16, tag="xt")
                        nc.scalar.dma_start(t[:], xt_d[:, ncx, b])
                        xts.append(t)
                    if ncx == 0:
                        nc.scalar.dma_start(wot_sb[:], wot_d[:])

                    for wsel, w4 in (("q", wq_d), ("k", wk_d)):
                        ps = [projps.tile([P, 512], F32, tag="proj",
                                          name=f"{wsel}_ps{ncx}_{i}") for i in range(HPC)]
                        for kb in range(KCH // 4):
                            w_t = wp.tile([P, 4, CW], BF16, tag=f"w{wsel}")
                            nc.sync.dma_start(w_t[:], w4[:, kb])
                            for ki in range(4):
                                k = 4 * kb + ki
                                rhs = xts[k // 8][:, k % 8, :]
                                for m in range(HPC):
                                    nc.tensor.matmul(
                                        ps[m][:], lhsT=w_t[:, ki, m * HD:(m + 1) * HD],
                                        rhs=rhs, start=(k == 0), stop=(k == KCH - 1))
                        dst = qt if wsel == "q" else kt
                        for m in range(HPC):
                            rope(ps[m], dst[m][:, ncx * 512:(ncx + 1) * 512], ncx)

                    v_ps = [projps.tile([P, 512], F32, tag="proj",
                                        name=f"v_ps{ncx}_{i}") for i in range(4)]
                    for kb in range(KCH // 4):
                        w_t = wp.tile([P, 4, CW], BF16, tag="wv")
                        nc.sync.dma_start(w_t[:], wv_d[:, kb])
                        for ki in range(4):
                            k = 4 * kb + ki
                            for t in range(4):
                                nc.tensor.matmul(
                                    v_ps[t][:],
                                    lhsT=xts[k // 8][:, k % 8, t * P:(t + 1) * P],
                                    rhs=w_t[:, ki, :], start=(k == 0), stop=(k == KCH - 1))
                    for t in range(4):
                        nc.vector.tensor_copy(v_sb[:, ncx * 4 + t, :], v_ps[t][:])

            # ---------------- Phase 2+3: attention with interleaved out-proj ----------------
            with tc.tile_pool(name="avtsp", bufs=8) as avtsp, \
                 tc.tile_pool(name="probsp", bufs=6) as probsp, \
                 tc.tile_pool(name="saccp", bufs=2) as saccp, \
                 tc.tile_pool(name="recp", bufs=2) as recp, \
                 tc.tile_pool(name="osbp", bufs=3) as osbp, \
                 tc.tile_pool(name="stps", bufs=2, space="PSUM") as stps, \
                 tc.tile_pool(name="avtps", bufs=2, space="PSUM") as avtps, \
                 tc.tile_pool(name="outps", bufs=2, space="PSUM") as outps:

                avt_all = [[None] * HPC for _ in range(NCHUNKS)]
                pending_out = deque()
                pending_epi = [None]

                def emit_outproj_unit(qc_, u):
                    qs, hc = u % 4, u // 4
                    o_ps = outps.tile([P, 512], F32, tag="o", name=f"o_ps{qc_}_{u}")
                    for h in range(HPC):
                        nc.tensor.matmul(
                            o_ps[:],
                            lhsT=avt_all[qc_][h][:, qs * P:(qs + 1) * P],
                            rhs=wot_sb[:, h, hc * 512:(hc + 1) * 512],
                            start=(h == 0), stop=(h == HPC - 1))
                    o_sb = osbp.tile([P, 512], BF16, tag="osb")
                    nc.vector.tensor_copy(o_sb[:], o_ps[:])
                    nc.scalar.dma_start(
                        out_d[qc_ * 512 + qs * P: qc_ * 512 + (qs + 1) * P,
                              hc * 512:(hc + 1) * 512],
                        o_sb[:])

                for qc in range(NCHUNKS):
                    qsl = slice(qc * 512, (qc + 1) * 512)
                    for h in range(HPC):
                        avt_ps = avtps.tile([P, 512], F32, tag="avt",
                                            name=f"avt{qc}_{h}")
                        sacc = saccp.tile([P, 512], F32R, tag="sacc",
                                          name=f"sacc{qc}_{h}")
                        nkt = 4 * (qc + 1)
                        for ktb in range(nkt // 2):
                            if pending_epi[0] is not None:
                                pending_epi[0]()
                                pending_epi[0] = None
                            st2 = stps.tile([P, 2, 512], F32, tag="st")
                            for u2 in range(2):
                                kti = 2 * ktb + u2
                                nc.tensor.matmul(
                                    st2[:, u2, :], lhsT=kt[h][:, kti * P:(kti + 1) * P],
                                    rhs=qt[h][:, qsl], start=True, stop=True)
                            probs2 = probsp.tile([P, 2, 512], F32R, tag="probs")
                            nc.scalar.activation(probs2[:], st2[:],
                                                 mybir.ActivationFunctionType.Exp,
                                                 scale=EXP_SCALE)
                            j = 2 * ktb - 4 * qc
                            if j >= 0:
                                nc.vector.tensor_mul(probs2[:], probs2[:],
                                                     masks_sb[:, j:j + 2, :])
                            if ktb == 0:
                                nc.gpsimd.tensor_add(sacc[:], probs2[:, 0, :],
                                                     probs2[:, 1, :])
                            else:
                                nc.gpsimd.tensor_add(sacc[:], sacc[:], probs2[:, 0, :])
                                nc.gpsimd.tensor_add(sacc[:], sacc[:], probs2[:, 1, :])
                            for u2 in range(2):
                                kti = 2 * ktb + u2
                                nc.tensor.matmul(
                                    avt_ps[:], lhsT=v_sb[:, kti, h * HD:(h + 1) * HD],
                                    rhs=probs2[:, u2, :],
                                    start=(kti == 0), stop=(kti == nkt - 1))
                            for _ in range(3):
                                if pending_out:
                                    emit_outproj_unit(*pending_out.popleft())
                        # Partition-reduce the folded probs sums; the all-ones
                        # stationary replicates the column sums to every
                        # partition, so no separate broadcast is needed.
                        sums_ps = outps.tile([P, 512], F32, tag="o",
                                             name=f"sums{qc}_{h}")
                        nc.tensor.matmul(sums_ps[:], lhsT=ones_sb[:], rhs=sacc[:],
                                         start=True, stop=True)
                        avs = avtsp.tile([P, 512], BF16, tag="avts",
                                         name=f"avts{qc}_{h}")
                        avt_all[qc][h] = avs

                        def epi(sums_ps=sums_ps, avt_ps=avt_ps, avs=avs):
                            rec = recp.tile([P, 512], F32, tag="rec")
                            nc.vector.reciprocal_approx_fast(out=rec[:], in_=sums_ps[:])
                            nc.vector.tensor_mul(avs[:], avt_ps[:], rec[:])

                        pending_epi[0] = epi
                    pending_out.extend((qc, u) for u in range(32))

                if pending_epi[0] is not None:
                    pending_epi[0]()
                    pending_epi[0] = None
                while pending_out:
                    emit_outproj_unit(*pending_out.popleft())

    nc.compile()
    return nc


def _fold(W, A, B):
    """Fold LoRA + its half/interleave permutation into the base weight."""
    BA = (B.astype(np.float64) @ A.astype(np.float64)) * LORA_SCALING
    j = np.arange(H)
    g = np.where(j < H // 2, 2 * j, 2 * (j - H // 2) + 1)
    return (W.astype(np.float64) + BA[g, :]).astype(np.float32)


def _host_consts():
    inv_freq = (1.0 / (10000.0 ** (np.arange(0, HD, 2, dtype=np.float32) / HD))).astype(np.float32)
    freqs = np.arange(S, dtype=np.float32)[:, None] * inv_freq[None, :]   # (S, 64)
    cs = np.concatenate([np.cos(freqs).T, np.sin(freqs).T], axis=0).astype(np.float32)  # (128, S)
    p = np.arange(P)[:, None, None]
    jj = np.arange(4)[None, :, None]
    f = np.arange(512)[None, None, :]
    masks = (jj * P + p <= f).astype(np.float32)  # (128, 4, 512)
    ones = np.ones((P, P), dtype=np.float32)
    return cs, masks, ones


def _pack_x(x):
    """x: (S, H) f32 -> xt (P, NCHUNKS, 4, 8, 512) bf16, per-partition contiguous."""
    xT = np.ascontiguousarray(x.T)                        # (H, S)
    t = xT.reshape(4, 8, P, NCHUNKS, 512)                 # (b, ki, p, ncx, s)
    return np.ascontiguousarray(t.transpose(2, 3, 0, 1, 4)).astype(ml_dtypes.bfloat16)


def _pack_w(Wc):
    """Wc: (CW, H) slice of folded weight -> (P, 8, 4, CW) bf16."""
    wT = np.ascontiguousarray(Wc.T)                       # (H, CW)
    t = wT.reshape(8, 4, P, CW)                           # (kb, ki, p, m)
    return np.ascontiguousarray(t.transpose(2, 0, 1, 3)).astype(ml_dtypes.bfloat16)


def _pack_wot(Wo_cols):
    """Wo[:, cols].T: (CW, H) -> (P, HPC, H) bf16."""
    t = Wo_cols.reshape(HPC, P, H)                        # (h, p, n)
    return np.ascontiguousarray(t.transpose(1, 0, 2)).astype(ml_dtypes.bfloat16)


def kernel(hidden_states, Wq, Wk, Wv, Wo, Aq, Bq, Ak, Bk, Av, Bv):
    if "nc" not in _CACHE:
        _CACHE["nc"] = _build()
    nc = _CACHE["nc"]

    x = np.ascontiguousarray(np.asarray(hidden_states, dtype=np.float32)[0])  # (S, H)
    xt_bf = _pack_x(x)

    Wq_eff = _fold(np.asarray(Wq), np.asarray(Aq), np.asarray(Bq))
    Wk_eff = _fold(np.asarray(Wk), np.asarray(Ak), np.asarray(Bk))
    Wv_eff = _fold(np.asarray(Wv), np.asarray(Av), np.asarray(Bv))
    Wo_np = np.asarray(Wo, dtype=np.float32)

    cs, masks, ones = _host_consts()

    in_maps = []
    for c in range(NCORES):
        cols = slice(CW * c, CW * (c + 1))
        in_maps.append({
            "xt": xt_bf,
            "wq": _pack_w(Wq_eff[cols]),
            "wk": _pack_w(Wk_eff[cols]),
            "wv": _pack_w(Wv_eff[cols]),
            "wot": _pack_wot(np.ascontiguousarray(Wo_np[:, cols].T)),
            "cs": cs,
            "masks": masks,
            "ones": ones,
        })
    _CACHE["in_maps"] = in_maps

    res = bass_utils.run_bass_kernel_spmd(nc, in_maps, core_ids=list(range(NCORES)))
    acc = np.zeros((S, H), dtype=np.float64)
    for c in range(NCORES):
        acc += np.asarray(res.results[c]["out"]).astype(np.float64)
    return acc.astype(np.float32)[None]


# revision 10
# speedup vs baseline: 1.1411x; 1.1411x over previous
"""Trainium2 Bass kernel for ConvertedLlamaAttention (LoRA q/k/v + RoPE + causal attention + out-proj).

Strategy: tensor-parallel over heads across 8 NeuronCores (4 heads/core).
All device matmuls run in "transposed" layouts so no on-device transposes are
needed anywhere:
  - Q^T, K^T computed as W^T-stationary matmuls (head_dim on partitions),
  - V computed in natural layout (seq on partitions) from the same X^T tiles,
  - scores computed transposed (S^T = K^T^T-slices @ Q^T); softmax denominators
    are accumulated on the vector engine (per-k-tile folds) and partition-
    reduced with a single all-ones matmul per (q-chunk, head) whose output is
    replicated across partitions (no separate broadcast matmul),
  - A*V consumes V in natural layout; out-proj consumes A*V^T as stationary.
All stationary matmul operands are bf16 (enables fast weight load); host-side
tensor layouts are per-partition contiguous so DMA descriptors are large.
Out-proj is interleaved into the attention stream (2 tiles per k-tile-pair) so
the tensor engine never starves and output DMA is spread across the kernel.
LoRA (incl. the half-interleave) is folded into the weights on the host.
Each core emits a partial (2048, 4096) bf16 output (row-parallel Wo); the host
sums in fp64.
"""
import sys

for _p in ("/opt/trn_rl_repo", "/root/.axon_site/_ro/trn_rl_repo"):
    if _p not in sys.path:
        sys.path.insert(0, _p)

from collections import deque

import numpy as np
import ml_dtypes

import concourse.bass as bass  # noqa: F401  (registers types)
import concourse.mybir as mybir
import concourse.tile as tile
from concourse import bacc, bass_utils

F32 = mybir.dt.float32
F32R = mybir.dt.float32r
BF16 = mybir.dt.bfloat16

H = 4096          # hidden
S = 2048          # sequence
P = 128           # partitions
HD = 128          # head dim
NCORES = 8
HPC = 4           # heads per core
CW = HPC * HD     # per-core width of q/k/v/attn dims = 512
NCHUNKS = 4       # seq chunks of 512
KCH = H // P      # 32 hidden chunks
LORA_SCALING = 2.0
EXP_SCALE = float(1.0 / np.sqrt(HD))

_CACHE = {}


def _build():
    nc = bacc.Bacc("TRN2", target_bir_lowering=False, debug=False, num_devices=NCORES)

    # Host-prearranged, per-partition-contiguous layouts.
    xt_d = nc.declare_dram_parameter("xt", [P, NCHUNKS, 4, 8, 512], BF16, isOutput=False)
    wq_d = nc.declare_dram_parameter("wq", [P, 8, 4, CW], BF16, isOutput=False)
    wk_d = nc.declare_dram_parameter("wk", [P, 8, 4, CW], BF16, isOutput=False)
    wv_d = nc.declare_dram_parameter("wv", [P, 8, 4, CW], BF16, isOutput=False)
    wot_d = nc.declare_dram_parameter("wot", [P, HPC, H], BF16, isOutput=False)
    cs_d = nc.declare_dram_parameter("cs", [P, S], F32, isOutput=False)
    masks_d = nc.declare_dram_parameter("masks", [P, 4, 512], F32R, isOutput=False)
    ones_d = nc.declare_dram_parameter("ones", [P, P], F32R, isOutput=False)
    out_d = nc.declare_dram_parameter("out", [S, H], BF16, isOutput=True)

    with tile.TileContext(nc) as tc:
        with tc.tile_pool(name="persist", bufs=1) as pp:
            qt = [pp.tile([P, S], BF16, tag=f"qt{h}", name=f"qt{h}") for h in range(HPC)]
            kt = [pp.tile([P, S], BF16, tag=f"kt{h}", name=f"kt{h}") for h in range(HPC)]
            v_sb = pp.tile([P, S // P, CW], F32R, tag="v")   # (128, 16, 512)
            cs_sb = pp.tile([P, S], F32, tag="cs")
            ones_sb = pp.tile([P, P], F32R, tag="ones")
            wot_sb = pp.tile([P, HPC, H], BF16, tag="wot")
            masks_sb = pp.tile([P, 4, 512], F32R, tag="masks")


            def rope(qp, dest, ncx):
                sl = slice(ncx * 512, (ncx + 1) * 512)
                t1 = ropep.tile([P, 512], F32, tag="r1")
                t2 = ropep.tile([P, 512], F32, tag="r2")
                # dest[0:64]  = q1*cos - q2*sin ; dest[64:] = q1*sin + q2*cos
                nc.vector.tensor_mul(t1[0:64], qp[0:64], cs_sb[0:64, sl])
                nc.vector.tensor_mul(t2[0:64], qp[64:128], cs_sb[64:128, sl])
                nc.vector.tensor_sub(dest[0:64], t1[0:64], t2[0:64])
                nc.vector.tensor_mul(t1[64:128], qp[0:64], cs_sb[64:128, sl])
                nc.vector.tensor_mul(t2[64:128], qp[64:128], cs_sb[0:64, sl])
                nc.vector.tensor_add(dest[64:128], t1[64:128], t2[64:128])

            # ---------------- Phase 1: Q^T, K^T, V projections ----------------
            with tc.tile_pool(name="xtp", bufs=6) as xtp, \
                 tc.tile_pool(name="wp", bufs=4) as wp, \
                 tc.tile_pool(name="ropep", bufs=2) as ropep, \
                 tc.tile_pool(name="projps", bufs=8, space="PSUM") as projps:
                for ncx in range(NCHUNKS):
                    xts = []
                    for b in range(4):
                        t = xtp.tile([P, 8, 512], BF16, tag="xt")
                        nc.scalar.dma_start(t[:], xt_d[:, ncx, b])
                        xts.append(t)
                    if ncx == 0:
                        nc.scalar.dma_start(cs_sb[:], cs_d[:])
                        nc.scalar.dma_start(ones_sb[:], ones_d[:])
                        nc.scalar.dma_start(masks_sb[:], masks_d[:])
                    elif ncx == 1:
                        nc.scalar.dma_start(wot_sb[:], wot_d[:])

                    for wsel, w4 in (("q", wq_d), ("k", wk_d)):
                        ps = [projps.tile([P, 512], F32, tag="proj",
                                          name=f"{wsel}_ps{ncx}_{i}") for i in range(HPC)]
                        for kb2 in range(KCH // 8):
                            w_t = wp.tile([P, 2, 4, CW], BF16, tag="w")
                            nc.sync.dma_start(w_t[:], w4[:, 2 * kb2:2 * kb2 + 2])
                            for ki in range(8):
                                k = 8 * kb2 + ki
                                rhs = xts[k // 8][:, k % 8, :]
                                for m in range(HPC):
                                    nc.tensor.matmul(
                                        ps[m][:],
                                        lhsT=w_t[:, ki // 4, ki % 4, m * HD:(m + 1) * HD],
                                        rhs=rhs, start=(k == 0), stop=(k == KCH - 1))
                        dst = qt if wsel == "q" else kt
                        for m in range(HPC):
                            rope(ps[m], dst[m][:, ncx * 512:(ncx + 1) * 512], ncx)

                    v_ps = [projps.tile([P, 512], F32, tag="proj",
                                        name=f"v_ps{ncx}_{i}") for i in range(4)]
                    for kb2 in range(KCH // 8):
                        w_t = wp.tile([P, 2, 4, CW], BF16, tag="w")
                        nc.scalar.dma_start(w_t[:], wv_d[:, 2 * kb2:2 * kb2 + 2])
                        for ki in range(8):
                            k = 8 * kb2 + ki
                            for t in range(4):
                                nc.tensor.matmul(
                                    v_ps[t][:],
                                    lhsT=xts[k // 8][:, k % 8, t * P:(t + 1) * P],
                                    rhs=w_t[:, ki // 4, ki % 4, :],
                                    start=(k == 0), stop=(k == KCH - 1))
                    for t in range(4):
                        nc.vector.tensor_copy(v_sb[:, ncx * 4 + t, :], v_ps[t][:])

            # ---------------- Phase 2+3: attention with interleaved out-proj ----------------
            with tc.tile_pool(name="avtsp", bufs=8) as avtsp, \
                 tc.tile_pool(name="probsp", bufs=6) as probsp, \
                 tc.tile_pool(name="saccp", bufs=2) as saccp, \
                 tc.tile_pool(name="recp", bufs=2) as recp, \
                 tc.tile_pool(name="osbp", bufs=3) as osbp, \
                 tc.tile_pool(name="stps", bufs=2, space="PSUM") as stps, \
                 tc.tile_pool(name="avtps", bufs=2, space="PSUM") as avtps, \
                 tc.tile_pool(name="outps", bufs=2, space="PSUM") as outps:

                avt_all = [[None] * HPC for _ in range(NCHUNKS)]
                pending_out = deque()
                pending_epi = [None]

                ocnt = [0]

                def emit_outproj_unit(qc_, u):
                    qs, hc = u % 4, u // 4
                    o_ps = outps.tile([P, 512], F32, tag="o", name=f"o_ps{qc_}_{u}")
                    for h in range(HPC):
                        nc.tensor.matmul(
                            o_ps[:],
                            lhsT=avt_all[qc_][h][:, qs * P:(qs + 1) * P],
                            rhs=wot_sb[:, h, hc * 512:(hc + 1) * 512],
                            start=(h == 0), stop=(h == HPC - 1))
                    o_sb = osbp.tile([P, 512], BF16, tag="osb")
                    ocnt[0] += 1
                    if ocnt[0] % 2 == 0:
                        nc.scalar.activation(o_sb[:], o_ps[:],
                                             mybir.ActivationFunctionType.Copy)
                    else:
                        nc.vector.tensor_copy(o_sb[:], o_ps[:])
                    nc.sync.dma_start(
                        out_d[qc_ * 512 + qs * P: qc_ * 512 + (qs + 1) * P,
                              hc * 512:(hc + 1) * 512],
                        o_sb[:])

                for qc in range(NCHUNKS):
                    qsl = slice(qc * 512, (qc + 1) * 512)
                    for h in range(HPC):
                        avt_ps = avtps.tile([P, 512], F32, tag="avt",
                                            name=f"avt{qc}_{h}")
                        sacc = saccp.tile([P, 512], F32R, tag="sacc",
                                          name=f"sacc{qc}_{h}")
                        nkt = 4 * (qc + 1)
                        for ktb in range(nkt // 2):
                            if pending_epi[0] is not None:
                                pending_epi[0]()
                                pending_epi[0] = None
                            st2 = stps.tile([P, 2, 512], F32, tag="st")
                            for u2 in range(2):
                                kti = 2 * ktb + u2
                                nc.tensor.matmul(
                                    st2[:, u2, :], lhsT=kt[h][:, kti * P:(kti + 1) * P],
                                    rhs=qt[h][:, qsl], start=True, stop=True)
                            probs2 = probsp.tile([P, 2, 512], F32R, tag="probs")
                            nc.scalar.activation(probs2[:], st2[:],
                                                 mybir.ActivationFunctionType.Exp,
                                                 scale=EXP_SCALE)
                            j = 2 * ktb - 4 * qc
                            if j >= 0:
                                nc.vector.tensor_mul(probs2[:], probs2[:],
                                                     masks_sb[:, j:j + 2, :])
                            if ktb == 0:
                                nc.vector.tensor_add(sacc[:], probs2[:, 0, :],
                                                     probs2[:, 1, :])
                            else:
                                nc.vector.tensor_add(sacc[:], sacc[:], probs2[:, 0, :])
                                nc.vector.tensor_add(sacc[:], sacc[:], probs2[:, 1, :])
                            for u2 in range(2):
                                kti = 2 * ktb + u2
                                nc.tensor.matmul(
                                    avt_ps[:], lhsT=v_sb[:, kti, h * HD:(h + 1) * HD],
                                    rhs=probs2[:, u2, :],
                                    start=(kti == 0), stop=(kti == nkt - 1))
                            for _ in range(3):
                                if pending_out:
                                    emit_outproj_unit(*pending_out.popleft())
                        # Partition-reduce the folded probs sums; the all-ones
                        # stationary replicates the column sums to every
                        # partition, so no separate broadcast is needed.
                        sums_ps = outps.tile([P, 512], F32, tag="o",
                                             name=f"sums{qc}_{h}")
                        nc.tensor.matmul(sums_ps[:], lhsT=ones_sb[:], rhs=sacc[:],
                                         start=True, stop=True)
                        avs = avtsp.tile([P, 512], BF16, tag="avts",
                                         name=f"avts{qc}_{h}")
                        avt_all[qc][h] = avs

                        def epi(sums_ps=sums_ps, avt_ps=avt_ps, avs=avs):
                            rec = recp.tile([P, 512], F32, tag="rec")
                            nc.vector.reciprocal_approx_fast(out=rec[:], in_=sums_ps[:])
                            nc.vector.tensor_mul(avs[:], avt_ps[:], rec[:])

                        pending_epi[0] = epi
                    pending_out.extend((qc, u) for u in range(32))

                if pending_epi[0] is not None:
                    pending_epi[0]()
                    pending_epi[0] = None
                while pending_out:
                    emit_outproj_unit(*pending_out.popleft())

    nc.compile()
    return nc


def _fold(W, A, B):
    """Fold LoRA + its half/interleave permutation into the base weight."""
    BA = (B.astype(np.float64) @ A.astype(np.float64)) * LORA_SCALING
    j = np.arange(H)
    g = np.where(j < H // 2, 2 * j, 2 * (j - H // 2) + 1)
    return (W.astype(np.float64) + BA[g, :]).astype(np.float32)


def _host_consts():
    inv_freq = (1.0 / (10000.0 ** (np.arange(0, HD, 2, dtype=np.float32) / HD))).astype(np.float32)
    freqs = np.arange(S, dtype=np.float32)[:, None] * inv_freq[None, :]   # (S, 64)
    cs = np.concatenate([np.cos(freqs).T, np.sin(freqs).T], axis=0).astype(np.float32)  # (128, S)
    p = np.arange(P)[:, None, None]
    jj = np.arange(4)[None, :, None]
    f = np.arange(512)[None, None, :]
    masks = (jj * P + p <= f).astype(np.float32)  # (128, 4, 512)
    ones = np.ones((P, P), dtype=np.float32)
    return cs, masks, ones


def _pack_x(x):
    """x: (S, H) f32 -> xt (P, NCHUNKS, 4, 8, 512) bf16, per-partition contiguous."""
    xT = np.ascontiguousarray(x.T)                        # (H, S)
    t = xT.reshape(4, 8, P, NCHUNKS, 512)                 # (b, ki, p, ncx, s)
    return np.ascontiguousarray(t.transpose(2, 3, 0, 1, 4)).astype(ml_dtypes.bfloat16)


def _pack_w(Wc):
    """Wc: (CW, H) slice of folded weight -> (P, 8, 4, CW) bf16."""
    wT = np.ascontiguousarray(Wc.T)                       # (H, CW)
    t = wT.reshape(8, 4, P, CW)                           # (kb, ki, p, m)
    return np.ascontiguousarray(t.transpose(2, 0, 1, 3)).astype(ml_dtypes.bfloat16)


def _pack_wot(Wo_cols):
    """Wo[:, cols].T: (CW, H) -> (P, HPC, H) bf16."""
    t = Wo_cols.reshape(HPC, P, H)                        # (h, p, n)
    return np.ascontiguousarray(t.transpose(1, 0, 2)).astype(ml_dtypes.bfloat16)


def kernel(hidden_states, Wq, Wk, Wv, Wo, Aq, Bq, Ak, Bk, Av, Bv):
    if "nc" not in _CACHE:
        _CACHE["nc"] = _build()
    nc = _CACHE["nc"]

    x = np.ascontiguousarray(np.asarray(hidden_states, dtype=np.float32)[0])  # (S, H)
    xt_bf = _pack_x(x)

    Wq_eff = _fold(np.asarray(Wq), np.asarray(Aq), np.asarray(Bq))
    Wk_eff = _fold(np.asarray(Wk), np.asarray(Ak), np.asarray(Bk))
    Wv_eff = _fold(np.asarray(Wv), np.asarray(Av), np.asarray(Bv))
    Wo_np = np.asarray(Wo, dtype=np.float32)

    cs, masks, ones = _host_consts()

    in_maps = []
    for c in range(NCORES):
        cols = slice(CW * c, CW * (c + 1))
        in_maps.append({
            "xt": xt_bf,
            "wq": _pack_w(Wq_eff[cols]),
            "wk": _pack_w(Wk_eff[cols]),
            "wv": _pack_w(Wv_eff[cols]),
            "wot": _pack_wot(np.ascontiguousarray(Wo_np[:, cols].T)),
            "cs": cs,
            "masks": masks,
            "ones": ones,
        })
    _CACHE["in_maps"] = in_maps

    res = bass_utils.run_bass_kernel_spmd(nc, in_maps, core_ids=list(range(NCORES)))
    acc = np.zeros((S, H), dtype=np.float64)
    for c in range(NCORES):
        acc += np.asarray(res.results[c]["out"]).astype(np.float64)
    return acc.astype(np.float32)[None]
